# revision 27
# baseline (speedup 1.0000x reference)
"""Trainium2 Bass kernel for nn_Attention_29738353557815.

8-way tensor-parallel over heads, restructured for early-collective overlap:
  - core c owns q-heads {2c, 2c+1} and kv-head c//2 (k/v duplicated per pair)
  - phase A: per t-chunk j: k(j), v(j), q0(j), g0(j) projections + h0
    attention -> AllToAll(h0) fires at ~55% of the kernel
  - phase C: q1/g1 projections + h1 attention overlap A2A1; AllToAll(h1)
  - phase D: o-proj h0 ht-blocks run inside the A2A2 window, then h1 blocks;
    m0 output half written while m1 still accumulates
  - the Pool queue carries ONLY the two collectives: a CollectiveCompute
    occupies its issuing engine for the full ~28us, so causal masking uses a
    host-precomputed causrel table via scalar_tensor_tensor on DVE (same
    pattern as the segment mask), and rowsum partial adds run on DVE too
  - DMAs are consolidated (the issuing engine's SEQ is held for the whole
    transfer): hidden^T in 4 per-chunk [P, DT, 512] fp16 DMAs, single-DMA
    weight packs, one staging DMA per (head, chunk), one gather DMA per head
  - all DMA'd operands fp16; rel-err budget is 2e-2, fp16 keeps us ~1e-3
  - softmax: exp with a -4ln2 bias (cancels between numerator/denominator)
    so fp16 probability tiles cannot overflow; rowsum via DVE partial-sum
    adds + one ones-matmul per (h, chunk) instead of a matmul per s-tile
  - rms-norm folded into ln/exp on ACT; rope tables (cos/sin * sqrt(scale))
    host-precomputed in [hd, T] fp16 layout; rotate-half via half-tile
    tensor_tensor ops against a half-swapped sin table
  - invalid (s,t) tiles are skipped entirely at build time (segment sparsity)
"""
import sys

if "/opt/trn_rl_repo" not in sys.path:
    sys.path.insert(0, "/opt/trn_rl_repo")

import numpy as np

import concourse.bass as bass
from concourse import bacc
import concourse.mybir as mybir
import concourse.tile as tile
from concourse.bass_utils import run_bass_kernel_spmd
from concourse.masks import make_identity

F32 = mybir.dt.float32
F32R = mybir.dt.float32r
F16 = mybir.dt.float16
AF = mybir.ActivationFunctionType
OP = mybir.AluOpType

B, T, D = 1, 2048, 2048
NH, NKV, HD = 16, 4, 128
EPS = 1e-6
SCALE = HD ** -0.5
NCORES = 8
P = 128
NJ = T // 512      # 4 t-chunks of 512
NT = T // P        # 16 s-tiles of 128
DT = D // P        # 16 contraction tiles
TSL = T // NCORES  # 256 output rows per core
EXP_BIAS = -2.772588722239781  # -4*ln2: shifts exp into fp16-safe range

_program_cache: dict = {}


def _tile_flags(seg_end: np.ndarray):
    """Per (s-tile i, t-chunk j): (skip, needs_causal, needs_seg)."""
    flags = []
    for i in range(NT):
        smin, smax = P * i, P * i + P - 1
        se_lo = int(seg_end[smin])
        se_hi = int(seg_end[smax])
        row = []
        for j in range(NJ):
            t0, t1 = 512 * j, 512 * j + 511
            skip = (t1 < smin) or (t0 >= se_hi)
            causal = (not skip) and (t0 < smax)
            segm = (not skip) and (t1 >= se_lo)
            row.append((skip, causal, segm))
        flags.append(row)
    return tuple(tuple(r) for r in flags)


def _build_program(key, use_collective=True):
    flags, unit_w = key
    nc = bacc.Bacc("TRN2", target_bir_lowering=False, debug=False,
                   num_devices=NCORES)

    hT_d = nc.dram_tensor("hT", [P, DT, T], F16, kind="ExternalInput")
    wqg_d = nc.dram_tensor("wqg", [P, DT, 512], F16, kind="ExternalInput")
    wkv_d = nc.dram_tensor("wkv", [P, DT, 256], F16, kind="ExternalInput")
    wo_d = nc.dram_tensor("wo", [P, NT, 2048], F16, kind="ExternalInput")
    tblq_d = nc.dram_tensor("tblq", [P, 2, T], F16, kind="ExternalInput")
    if not unit_w:
        wqk_d = nc.dram_tensor("wqk", [P, 2], F32, kind="ExternalInput")
    iota_d = nc.dram_tensor("iota", [P, 512], F16, kind="ExternalInput")
    rel_d = nc.dram_tensor("rel", [P, 2, NT, NJ], F32, kind="ExternalInput")
    out_d = nc.dram_tensor("out", [TSL, D], F32, kind="ExternalOutput")

    with tile.TileContext(nc) as tc:
        with (
            tc.tile_pool(name="consts", bufs=1) as consts,
            tc.tile_pool(name="perm", bufs=1) as perm,
            tc.tile_pool(name="hw", bufs=4) as hw,
            tc.tile_pool(name="tmp", bufs=5) as tmp,
            tc.tile_pool(name="ptp", bufs=5) as ptp,
            tc.tile_pool(name="ps", bufs=1, space="PSUM") as psp,
            tc.tile_pool(name="dram", bufs=1, space="DRAM") as dram,
        ):
            # ---- constants ----
            wqg_sb = consts.tile([P, DT, 512], F16, tag="wqg", name="wqg")
            wkv_sb = consts.tile([P, DT, 256], F16, tag="wkv", name="wkv")
            tbq = consts.tile([P, 2, T], F16, tag="tbq", name="tbq")
            if not unit_w:
                wqk_sb = consts.tile([P, 2], F32)
                nc.sync.dma_start(wqk_sb[:], wqk_d[:])
            iota_sb = consts.tile([P, 512], F16)
            rel_sb = consts.tile([P, 2, NT, NJ], F32)
            ones_f32 = consts.tile([P, P], F32)
            nc.vector.memset(ones_f32[:], 1.0)
            ones_sb = consts.tile([P, P], F32R)
            nc.vector.tensor_copy(ones_sb[:], ones_f32[:])
            ones16 = consts.tile([P, P], F16)
            nc.vector.tensor_copy(ones16[:], ones_f32[:])
            ident_f32 = consts.tile([P, P], F32)
            make_identity(nc, ident_f32[:])
            ident16 = consts.tile([P, P], F16)
            nc.vector.tensor_copy(ident16[:], ident_f32[:])
            eps_sb = consts.tile([P, 1], F32)
            nc.vector.memset(eps_sb[:], EPS)
            expb_sb = consts.tile([P, 1], F32)
            nc.vector.memset(expb_sb[:], EXP_BIAS)

            # ---- persistent activations ----
            qTr = [perm.tile([P, T], F16, tag=f"qTr{h}", name=f"qTr{h}")
                   for h in range(2)]
            kTr = perm.tile([P, T], F16, tag="kTr")
            gT = [perm.tile([P, T], F16, tag=f"gT{h}", name=f"gT{h}")
                  for h in range(2)]
            v_sb = perm.tile([P, NT, P], F16, tag="v_sb")
            ATall = [perm.tile([P, NCORES, TSL], F16, tag=f"ATall{h}",
                               name=f"ATall{h}") for h in range(2)]

            a2a_in = [dram.tile([NCORES * P, TSL], F16, name=f"a2a_in{h}")
                      for h in range(2)]
            a2a_in_r = [a.rearrange("(s r) t -> r s t", r=P) for a in a2a_in]
            a2a_out = [dram.tile([NCORES * P, TSL], F16, name=f"a2a_out{h}")
                       for h in range(2)]
            a2a_out_r = [a.rearrange("(s r) t -> r s t", r=P) for a in a2a_out]

            # hT chunk tiles [P, DT, 512], loaded one DMA per chunk
            hTc = []

            def hslice(dt, j):
                return hTc[j][:, dt, :]

            def _norm_rope(mm_ps, dest, tsl, which):
                """rms-norm + rope from PSUM [hd, 512] into fp16 dest.
                Norm applied first (qn = mm * rsv), so rope runs in fp16."""
                qpre = tmp.tile([P, 512], F16, tag="qpre", bufs=2)
                nc.vector.tensor_copy(qpre[:], mm_ps[:])
                q2 = tmp.tile([P, 512], F32R, tag="q2", bufs=2)
                nc.scalar.activation(q2[:], mm_ps[:], AF.Square)
                ssq_ps = psp.tile([P, 512], F32, tag="aux", bufs=1)
                nc.tensor.matmul(ssq_ps[:], ones_sb[:], q2[:],
                                 start=True, stop=True)
                rsv = tmp.tile([P, 512], F32, tag="rsv", bufs=2)
                nc.scalar.activation(rsv[:], ssq_ps[:], AF.Ln,
                                     scale=1.0 / HD, bias=eps_sb[:, 0:1])
                rsv16 = tmp.tile([P, 512], F16, tag="rsv16", bufs=2)
                nc.scalar.activation(rsv16[:], rsv[:], AF.Exp, scale=-0.5)
                if not unit_w:
                    nc.vector.tensor_scalar_mul(
                        rsv16[:], rsv16[:], wqk_sb[:, which:which + 1])
                qn = tmp.tile([P, 512], F16, tag="qn", bufs=2, name="qn")
                nc.vector.tensor_tensor(qn[:], qpre[:], rsv16[:], OP.mult)
                tcos = tmp.tile([P, 512], F16, tag="tcos", bufs=2)
                nc.vector.tensor_tensor(tcos[:], qn[:], tbq[:, 0, tsl],
                                        OP.mult)
                t2 = tmp.tile([P, 512], F16, tag="t2", bufs=2)
                # sin table halves pre-swapped host-side; rotate-half via
                # shifted-partition writes
                nc.vector.tensor_tensor(t2[0:64, :], qn[64:128, :],
                                        tbq[64:128, 1, tsl], OP.mult)
                nc.vector.tensor_tensor(t2[64:128, :], qn[0:64, :],
                                        tbq[0:64, 1, tsl], OP.mult)
                nc.vector.tensor_tensor(dest, tcos[:], t2[:], OP.add)

            def emit_k_proj(j):
                tsl = slice(512 * j, 512 * j + 512)
                mm_ps = psp.tile([P, 512], F32, tag="proj", bufs=2,
                                 name=f"kproj_{j}")
                for dt in range(DT):
                    nc.tensor.matmul(mm_ps[:], wkv_sb[:, dt, 0:128],
                                     hslice(dt, j),
                                     start=(dt == 0), stop=(dt == DT - 1))
                _norm_rope(mm_ps, kTr[:, tsl], tsl, 1)

            def emit_v_proj(j):
                """v projection in [hd, t], then PE-transpose to [t, hd]."""
                v_ps = psp.tile([P, 512], F32, tag="vps", bufs=1,
                                name=f"vproj_{j}")
                for dt in range(DT):
                    nc.tensor.matmul(v_ps[:], wkv_sb[:, dt, 128:256],
                                     hslice(dt, j),
                                     start=(dt == 0), stop=(dt == DT - 1))
                vtmp = tmp.tile([P, 512], F16, tag="vtmp", bufs=2)
                nc.vector.tensor_copy(vtmp[:], v_ps[:])
                for kk in range(4):
                    tt = 4 * j + kk
                    trp = psp.tile([P, P], F16, tag="aux", bufs=1)
                    nc.tensor.transpose(
                        trp[:], vtmp[:, 128 * kk:128 * kk + 128], ident16[:])
                    nc.vector.tensor_copy(v_sb[:, tt, :], trp[:])

            def emit_q_proj(h, j):
                tsl = slice(512 * j, 512 * j + 512)
                mm_ps = psp.tile([P, 512], F32, tag="proj", bufs=2,
                                 name=f"qproj_{h}_{j}")
                for dt in range(DT):
                    nc.tensor.matmul(mm_ps[:], wqg_sb[:, dt, 128 * h:128 * h + 128],
                                     hslice(dt, j),
                                     start=(dt == 0), stop=(dt == DT - 1))
                _norm_rope(mm_ps, qTr[h][:, tsl], tsl, 0)

            def emit_gate_proj(h, j):
                """gate projection -> gT[h] holds ln(1+e^-g)."""
                tsl = slice(512 * j, 512 * j + 512)
                mm_ps = psp.tile([P, 512], F32, tag="proj", bufs=2,
                                 name=f"gproj_{h}_{j}")
                for dt in range(DT):
                    nc.tensor.matmul(
                        mm_ps[:], wqg_sb[:, dt, 256 + 128 * h:384 + 128 * h],
                        hslice(dt, j),
                        start=(dt == 0), stop=(dt == DT - 1))
                eg = tmp.tile([P, 512], F32, tag="eg", bufs=2)
                nc.scalar.activation(eg[:], mm_ps[:], AF.Exp, scale=-1.0)
                nc.scalar.activation(gT[h][:, tsl], eg[:], AF.Ln, bias=1.0)

            def emit_attention(h, j):
                tsl = slice(512 * j, 512 * j + 512)
                valid = [i for i in range(NT) if not flags[i][j][0]]
                last = len(valid) - 1
                ot_ps = psp.tile([P, 512], F32, tag="acc", bufs=1,
                                 name=f"ot_{h}_{j}")
                acc = tmp.tile([P, 512], F16, tag="pacc", bufs=2,
                               name=f"pacc_{h}_{j}")
                for idx, i in enumerate(valid):
                    _, needs_c, needs_s = flags[i][j]
                    st_ps = psp.tile([P, 512], F32, tag="st", bufs=3,
                                     name=f"st_{h}_{j}_{i}")
                    nc.tensor.matmul(st_ps[:], kTr[:, P * i:P * i + P],
                                     qTr[h][:, tsl], start=True, stop=True)
                    pt = ptp.tile([P, 512], F16, tag="pt", name=f"pt_{h}_{j}_{i}")
                    nc.scalar.activation(pt[:], st_ps[:], AF.Exp,
                                         bias=expb_sb[:, 0:1])
                    if needs_c:
                        # causal: keep t >= 128*i - 512*j + s  (per-partition s)
                        nc.vector.scalar_tensor_tensor(
                            out=pt[:], in0=iota_sb[:],
                            scalar=rel_sb[:, 0, i, j:j + 1], in1=pt[:],
                            op0=OP.is_ge, op1=OP.mult)
                    if needs_s:
                        # segment: keep t < seg_end[s] - 512*j
                        nc.vector.scalar_tensor_tensor(
                            out=pt[:], in0=iota_sb[:],
                            scalar=rel_sb[:, 1, i, j:j + 1], in1=pt[:],
                            op0=OP.is_lt, op1=OP.mult)
                    nc.tensor.matmul(ot_ps[:], v_sb[:, i, :], pt[:],
                                     start=(idx == 0), stop=(idx == last))
                    if idx == 0:
                        nc.vector.tensor_copy(acc[:], pt[:])
                    else:
                        nc.vector.tensor_tensor(acc[:], acc[:], pt[:], OP.add)
                rs_ps = psp.tile([P, 512], F32, tag="st", bufs=3,
                                 name=f"rs_{h}_{j}")
                nc.tensor.matmul(rs_ps[:], ones16[:], acc[:],
                                 start=True, stop=True)

                # sig(g)/rowsum = exp(-(ln(1+e^-g) + ln(rowsum)));
                # the exp bias cancels between ot and rs
                sg = tmp.tile([P, 512], F16, tag="sg", bufs=2, name=f"sg_{h}_{j}")
                nc.scalar.activation(sg[:], rs_ps[:], AF.Ln)
                nc.vector.tensor_tensor(sg[:], sg[:], gT[h][:, tsl], OP.add)
                nc.scalar.activation(sg[:], sg[:], AF.Exp, scale=-1.0)
                atg = tmp.tile([P, 512], F16, tag="atg", bufs=2,
                               name=f"atg_{h}_{j}")
                nc.vector.tensor_tensor(atg[:], ot_ps[:], sg[:], OP.mult)
                # one staging DMA: [128, 2, 256] -> dram shards 2j, 2j+1
                nc.sync.dma_start(
                    a2a_in_r[h][:, 2 * j:2 * j + 2, :],
                    atg[:].rearrange("p (s t) -> p s t", t=256))

            # ================= phase A: h0 path =================
            # kv pass pipelines with the hT chunk DMAs; q0/g0/attention pass
            # follows once wqg and the rope tables have landed
            nc.sync.dma_start(wkv_sb[:], wkv_d[:])
            for j in range(NJ):
                t_ = hw.tile([P, DT, 512], F16, tag="hT", bufs=4,
                             name=f"hTc{j}")
                for s in range(2):
                    nc.sync.dma_start(t_[:, 8 * s:8 * s + 8, :],
                                      hT_d[:, 8 * s:8 * s + 8,
                                           512 * j:512 * j + 512])
                hTc.append(t_)
                if j == 1:  # wqg lands just before the q0 pass needs it
                    nc.sync.dma_start(wqg_sb[:], wqg_d[:])
                    nc.sync.dma_start(tbq[:], tblq_d[:])
                    nc.sync.dma_start(iota_sb[:], iota_d[:])
                    nc.sync.dma_start(rel_sb[:], rel_d[:])

            for j in range(NJ):
                emit_k_proj(j)
                emit_v_proj(j)
            for j in range(NJ):
                emit_q_proj(0, j)
                emit_gate_proj(0, j)
                emit_attention(0, j)
            if use_collective:
                nc.gpsimd.collective_compute(
                    "AllToAll", OP.bypass,
                    replica_groups=[list(range(NCORES))],
                    ins=[a2a_in[0][:].opt()], outs=[a2a_out[0][:].opt()])
            else:
                nc.sync.dma_start(a2a_out[0][:], a2a_in[0][:])

            # ================= phase C: h1 path =================
            # wo blocks 0-1 prefetch; rest stream in phase D
            wo_sb = []
            w_ = hw.tile([P, 2, 2048], F16, tag="wo", bufs=4, name="wo_0")
            nc.sync.dma_start(w_[:], wo_d[:, 0:2, :])
            wo_sb.append(w_)
            for j in range(NJ):
                emit_q_proj(1, j)
                emit_gate_proj(1, j)
                emit_attention(1, j)
            if use_collective:
                nc.gpsimd.collective_compute(
                    "AllToAll", OP.bypass,
                    replica_groups=[list(range(NCORES))],
                    ins=[a2a_in[1][:].opt()], outs=[a2a_out[1][:].opt()])
            else:
                nc.sync.dma_start(a2a_out[1][:], a2a_in[1][:])

            # ================= phase D: o-proj =================
            # 8 PSUM banks [m 0/1] x [Dc 0..3] accumulate over 16 ht blocks;
            # h0 blocks (ht 0..7) run inside the A2A2 window; within each
            # half m0 completes before m1 so its write overlaps m1 compute
            nc.scalar.dma_start(ATall[0][:], a2a_out_r[0][:])
            for w in range(1, 4):
                w_ = hw.tile([P, 2, 2048], F16, tag="wo", bufs=4,
                             name=f"wo_{w}")
                nc.scalar.dma_start(w_[:], wo_d[:, 2 * w:2 * w + 2, :])
                wo_sb.append(w_)
            ops_tags = ["proj", "proj", "vps", "st", "st", "st", "acc", "aux"]
            ops_bufs = {"proj": 2, "vps": 1, "st": 3, "acc": 1, "aux": 1}
            ops = []
            for m in range(2):
                for Dc in range(NJ):
                    tg = ops_tags[m * NJ + Dc]
                    ops.append(psp.tile([P, 512], F32, tag=tg,
                                        bufs=ops_bufs[tg],
                                        name=f"ops{m}_{Dc}"))
            for half in range(2):
                for m in range(2):
                    for ht in range(8 * half, 8 * half + 8):
                        at_ap = ATall[half][:, ht % 8,
                                            128 * m:128 * m + 128]
                        for Dc in range(NJ):
                            nc.tensor.matmul(
                                ops[m * NJ + Dc][:], at_ap,
                                wo_sb[ht // 2][:, ht % 2,
                                               512 * Dc:512 * Dc + 512],
                                start=(ht == 0), stop=(ht == NT - 1))
                if half == 0:
                    # emitted only now: an earlier emission shares a DMA
                    # completion lane with the h0 loads and its (A2A2-gated)
                    # count would hold back the half-0 matmul waits; same
                    # for the half-1 wo tiles, which also recycle wo slots
                    nc.scalar.dma_start(ATall[1][:], a2a_out_r[1][:])
                    for w in range(4, 8):
                        w_ = hw.tile([P, 2, 2048], F16, tag="wo", bufs=4,
                                     name=f"wo_{w}")
                        nc.scalar.dma_start(w_[:], wo_d[:, 2 * w:2 * w + 2, :])
                        wo_sb.append(w_)
            # write m0 while m1's second half still accumulates
            for m in range(2):
                for Dh in range(2):
                    o_sb = tmp.tile([P, 1024], F32, tag="osb", bufs=2,
                                    name=f"o_{m}_{Dh}")
                    for q in range(2):
                        nc.vector.tensor_copy(o_sb[:, 512 * q:512 * q + 512],
                                              ops[m * NJ + 2 * Dh + q][:])
                    nc.sync.dma_start(
                        out_d[128 * m:128 * m + 128,
                              1024 * Dh:1024 * Dh + 1024], o_sb[:])

    nc.compile()
    _dedupe_act_table_loads(nc)
    return nc


def _dedupe_act_table_loads(nc):
    """Bacc assigns Exp->exp_and_others and Ln->natural_log, inserting a
    ~2.7us table load at every Exp<->Ln alternation. All activation funcs
    this kernel uses (Exp, Ln, Square) live in the natural_log_exp_and_others
    set, so keep one load of that set and drop the rest."""
    from concourse.hw_specs import get_activation_tables
    tabs = list(get_activation_tables(nc.m.arch).items())
    nl_exp = next(i for i, (nm, funcs) in enumerate(tabs)
                  if nm == "natural_log_exp_and_others")
    used = {ins.func for bb in nc.main_func.blocks for ins in bb.instructions
            if isinstance(ins, mybir.InstActivation)}
    assert used <= tabs[nl_exp][1], f"funcs {used} not all in natural_log_exp"
    first = True
    for bb in nc.main_func.blocks:
        keep = []
        for ins in bb.instructions:
            if isinstance(ins, mybir.InstLoadActFuncSet):
                assert ins.sync_info is None or (
                    not ins.sync_info.on_wait and not ins.sync_info.on_update)
                if first:
                    ins.act_func_set_id = nl_exp
                    keep.append(ins)
                    first = False
                continue
            keep.append(ins)
        bb.instructions[:] = keep


def _host_prep(hidden_BTD, cos_BTK, sin_BTK, segment_ids_BT, position_ids_BT,
               wq, wk, wv, wo, q_norm_w, k_norm_w):
    hidden = np.ascontiguousarray(np.asarray(hidden_BTD, dtype=np.float32)[0])
    cos = np.asarray(cos_BTK, dtype=np.float32)[0]
    sin = np.asarray(sin_BTK, dtype=np.float32)[0]
    seg = np.asarray(segment_ids_BT)[0]
    pos = np.asarray(position_ids_BT)[0]
    wq = np.asarray(wq, dtype=np.float32)
    wk = np.asarray(wk, dtype=np.float32)
    wv = np.asarray(wv, dtype=np.float32)
    wo = np.asarray(wo, dtype=np.float32)
    q_norm_w = np.asarray(q_norm_w, dtype=np.float32)
    k_norm_w = np.asarray(k_norm_w, dtype=np.float32)

    assert np.array_equal(pos, np.arange(T, dtype=pos.dtype)), \
        "kernel assumes position_ids == arange"
    assert np.all(np.diff(seg) >= 0), "kernel assumes sorted segment ids"

    # [P, DT, T] fp16 partition-major hidden^T
    hT = np.ascontiguousarray(
        hidden.T.reshape(DT, P, T).transpose(1, 0, 2).astype(np.float16))
    sqrtS = np.float32(np.sqrt(SCALE))
    signv = np.where(np.arange(HD) < HD // 2, -1.0, 1.0).astype(np.float32)
    shuf = (np.arange(HD) + HD // 2) % HD

    cosw = (cos.T * sqrtS).astype(np.float32)
    sinw = (sin.T * signv[:, None] * sqrtS).astype(np.float32)
    sinswap = sinw[shuf]  # halves swapped: see rotate-half ops
    tblq = np.ascontiguousarray(
        np.stack([cosw, sinswap], axis=1).astype(np.float16))  # [P, 2, T]
    unit_w = bool(np.all(q_norm_w == 1.0) and np.all(k_norm_w == 1.0))
    wqk = np.ascontiguousarray(np.stack([q_norm_w, k_norm_w], axis=1))

    # prepack wo; ht-block order: all h0 head-blocks, then all h1
    perm = [2 * i + h for h in range(2) for i in range(NCORES)]
    wo_p = wo.reshape(NT, P, 2048)[perm].transpose(1, 0, 2)
    wo_p = np.ascontiguousarray(wo_p.astype(np.float16))

    seg_end = np.searchsorted(seg, seg, side="right").astype(np.int64)
    iota = np.broadcast_to(np.arange(512, dtype=np.float16), (P, 512)).copy()
    # rel[:, 0] causal: t-iota >= 128i - 512j + s; rel[:, 1] segment:
    # t-iota < seg_end[s] - 512j
    rel = np.zeros((P, 2, NT, NJ), dtype=np.float32)
    s_local = np.arange(P, dtype=np.float32)
    for i in range(NT):
        for j in range(NJ):
            rel[:, 0, i, j] = 128.0 * i - 512.0 * j + s_local
            rel[:, 1, i, j] = seg_end[P * i:P * i + P] - 512.0 * j

    in_maps = []
    for c in range(NCORES):
        h0, h1 = 2 * c, 2 * c + 1
        g = c // 2
        # per-core wq/gate pack: [q_h0 | q_h1 | g_h0 | g_h1] columns
        wqg = np.concatenate([
            wq[:, h0 * 256: h0 * 256 + 128],
            wq[:, h1 * 256: h1 * 256 + 128],
            wq[:, h0 * 256 + 128: h0 * 256 + 256],
            wq[:, h1 * 256 + 128: h1 * 256 + 256],
        ], axis=1)
        wqg_p = np.ascontiguousarray(
            wqg.reshape(DT, P, 512).transpose(1, 0, 2).astype(np.float16))
        wkv = np.concatenate([
            wk[:, g * 128:(g + 1) * 128], wv[:, g * 128:(g + 1) * 128]], axis=1)
        wkv_p = np.ascontiguousarray(
            wkv.reshape(DT, P, 256).transpose(1, 0, 2).astype(np.float16))
        m = {
            "hT": hT, "wqg": wqg_p, "wkv": wkv_p, "wo": wo_p,
            "tblq": tblq, "iota": iota, "rel": rel,
        }
        if not unit_w:
            m["wqk"] = wqk
        in_maps.append(m)
    return in_maps, seg_end, unit_w


def kernel(**inputs) -> np.ndarray:
    in_maps, seg_end, unit_w = _host_prep(**inputs)
    key = (_tile_flags(seg_end), unit_w)
    if key not in _program_cache:
        _program_cache[key] = _build_program(key)
    nc = _program_cache[key]
    res = run_bass_kernel_spmd(nc, in_maps, list(range(NCORES)))
    out = np.concatenate([res.results[c]["out"] for c in range(NCORES)], axis=0)
    return out[None].astype(np.float32)


# revision 30
# speedup vs baseline: 1.0052x; 1.0052x over previous
"""Trainium2 Bass kernel for nn_Attention_29738353557815.

8-way tensor-parallel over heads, restructured for early-collective overlap:
  - core c owns q-heads {2c, 2c+1} and kv-head c//2 (k/v duplicated per pair)
  - phase A: per t-chunk j: k(j), v(j), q0(j), g0(j) projections + h0
    attention -> AllToAll(h0) fires at ~55% of the kernel
  - phase C: q1/g1 projections + h1 attention overlap A2A1; AllToAll(h1)
  - phase D: o-proj h0 ht-blocks run inside the A2A2 window, then h1 blocks;
    m0 output half written while m1 still accumulates
  - the Pool queue carries ONLY the two collectives: a CollectiveCompute
    occupies its issuing engine for the full ~28us, so causal masking uses a
    host-precomputed causrel table via scalar_tensor_tensor on DVE (same
    pattern as the segment mask), and rowsum partial adds run on DVE too
  - DMAs are consolidated (the issuing engine's SEQ is held for the whole
    transfer): hidden^T in 4 per-chunk [P, DT, 512] fp16 DMAs, single-DMA
    weight packs, one staging DMA per (head, chunk), one gather DMA per head
  - all DMA'd operands fp16; rel-err budget is 2e-2, fp16 keeps us ~1e-3
  - softmax: exp with a -4ln2 bias (cancels between numerator/denominator)
    so fp16 probability tiles cannot overflow; rowsum via DVE partial-sum
    adds + one ones-matmul per (h, chunk) instead of a matmul per s-tile
  - rms-norm folded into ln/exp on ACT; rope tables (cos/sin * sqrt(scale))
    host-precomputed in [hd, T] fp16 layout; rotate-half via half-tile
    tensor_tensor ops against a half-swapped sin table
  - invalid (s,t) tiles are skipped entirely at build time (segment sparsity)
"""
import sys

if "/opt/trn_rl_repo" not in sys.path:
    sys.path.insert(0, "/opt/trn_rl_repo")

import numpy as np

import concourse.bass as bass
from concourse import bacc
import concourse.mybir as mybir
import concourse.tile as tile
from concourse.bass_utils import run_bass_kernel_spmd
from concourse.masks import make_identity

F32 = mybir.dt.float32
F32R = mybir.dt.float32r
F16 = mybir.dt.float16
AF = mybir.ActivationFunctionType
OP = mybir.AluOpType

B, T, D = 1, 2048, 2048
NH, NKV, HD = 16, 4, 128
EPS = 1e-6
SCALE = HD ** -0.5
NCORES = 8
P = 128
NJ = T // 512      # 4 t-chunks of 512
NT = T // P        # 16 s-tiles of 128
DT = D // P        # 16 contraction tiles
TSL = T // NCORES  # 256 output rows per core
EXP_BIAS = -2.772588722239781  # -4*ln2: shifts exp into fp16-safe range

_program_cache: dict = {}


def _tile_flags(seg_end: np.ndarray):
    """Per (s-tile i, t-chunk j): (skip, needs_causal, needs_seg)."""
    flags = []
    for i in range(NT):
        smin, smax = P * i, P * i + P - 1
        se_lo = int(seg_end[smin])
        se_hi = int(seg_end[smax])
        row = []
        for j in range(NJ):
            t0, t1 = 512 * j, 512 * j + 511
            skip = (t1 < smin) or (t0 >= se_hi)
            causal = (not skip) and (t0 < smax)
            segm = (not skip) and (t1 >= se_lo)
            row.append((skip, causal, segm))
        flags.append(row)
    return tuple(tuple(r) for r in flags)


def _build_program(key, use_collective=True):
    flags, unit_w = key
    nc = bacc.Bacc("TRN2", target_bir_lowering=False, debug=False,
                   num_devices=NCORES)

    hT_d = nc.dram_tensor("hT", [P, DT, T], F16, kind="ExternalInput")
    wqg_d = nc.dram_tensor("wqg", [P, DT, 512], F16, kind="ExternalInput")
    wkv_d = nc.dram_tensor("wkv", [P, DT, 256], F16, kind="ExternalInput")
    wo_d = nc.dram_tensor("wo", [P, NT, 2048], F16, kind="ExternalInput")
    tblq_d = nc.dram_tensor("tblq", [P, 2, T], F16, kind="ExternalInput")
    if not unit_w:
        wqk_d = nc.dram_tensor("wqk", [P, 2], F32, kind="ExternalInput")
    iota_d = nc.dram_tensor("iota", [P, 512], F16, kind="ExternalInput")
    rel_d = nc.dram_tensor("rel", [P, 2, NT, NJ], F32, kind="ExternalInput")
    out_d = nc.dram_tensor("out", [TSL, D], F32, kind="ExternalOutput")

    with tile.TileContext(nc) as tc:
        with (
            tc.tile_pool(name="consts", bufs=1) as consts,
            tc.tile_pool(name="perm", bufs=1) as perm,
            tc.tile_pool(name="hw", bufs=4) as hw,
            tc.tile_pool(name="tmp", bufs=5) as tmp,
            tc.tile_pool(name="ptp", bufs=5) as ptp,
            tc.tile_pool(name="ps", bufs=1, space="PSUM") as psp,
            tc.tile_pool(name="dram", bufs=1, space="DRAM") as dram,
        ):
            # ---- constants ----
            wqg_sb = consts.tile([P, DT, 512], F16, tag="wqg", name="wqg")
            wkv_sb = consts.tile([P, DT, 256], F16, tag="wkv", name="wkv")
            tbq = consts.tile([P, 2, T], F16, tag="tbq", name="tbq")
            if not unit_w:
                wqk_sb = consts.tile([P, 2], F32)
                nc.sync.dma_start(wqk_sb[:], wqk_d[:])
            iota_sb = consts.tile([P, 512], F16)
            rel_sb = consts.tile([P, 2, NT, NJ], F32)
            ones_f32 = consts.tile([P, P], F32)
            nc.vector.memset(ones_f32[:], 1.0)
            ones_sb = consts.tile([P, P], F32R)
            nc.vector.tensor_copy(ones_sb[:], ones_f32[:])
            ones16 = consts.tile([P, P], F16)
            nc.vector.tensor_copy(ones16[:], ones_f32[:])
            ident_f32 = consts.tile([P, P], F32)
            make_identity(nc, ident_f32[:])
            ident16 = consts.tile([P, P], F16)
            nc.vector.tensor_copy(ident16[:], ident_f32[:])
            eps_sb = consts.tile([P, 1], F32)
            nc.vector.memset(eps_sb[:], EPS)
            expb_sb = consts.tile([P, 1], F32)
            nc.vector.memset(expb_sb[:], EXP_BIAS)

            # ---- persistent activations ----
            qTr = [perm.tile([P, T], F16, tag=f"qTr{h}", name=f"qTr{h}")
                   for h in range(2)]
            kTr = perm.tile([P, T], F16, tag="kTr")
            gT = [perm.tile([P, T], F16, tag=f"gT{h}", name=f"gT{h}")
                  for h in range(2)]
            v_sb = perm.tile([P, NT, P], F16, tag="v_sb")
            ATall = [perm.tile([P, NCORES, TSL], F16, tag=f"ATall{h}",
                               name=f"ATall{h}") for h in range(2)]

            a2a_in = [dram.tile([NCORES * P, TSL], F16, name=f"a2a_in{h}")
                      for h in range(2)]
            a2a_in_r = [a.rearrange("(s r) t -> r s t", r=P) for a in a2a_in]
            a2a_out = [dram.tile([NCORES * P, TSL], F16, name=f"a2a_out{h}")
                       for h in range(2)]
            a2a_out_r = [a.rearrange("(s r) t -> r s t", r=P) for a in a2a_out]

            # hT chunk tiles [P, DT, 512], loaded one DMA per chunk
            hTc = []

            def hslice(dt, j):
                return hTc[j][:, dt, :]

            def _norm_rope(mm_ps, dest, tsl, which):
                """rms-norm + rope from PSUM [hd, 512] into fp16 dest.
                Norm applied first (qn = mm * rsv), so rope runs in fp16."""
                qpre = tmp.tile([P, 512], F16, tag="qpre", bufs=2)
                nc.vector.tensor_copy(qpre[:], mm_ps[:])
                q2 = tmp.tile([P, 512], F32R, tag="q2", bufs=2)
                nc.scalar.activation(q2[:], mm_ps[:], AF.Square)
                ssq_ps = psp.tile([P, 512], F32, tag="aux", bufs=1)
                nc.tensor.matmul(ssq_ps[:], ones_sb[:], q2[:],
                                 start=True, stop=True)
                rsv = tmp.tile([P, 512], F32, tag="rsv", bufs=2)
                nc.scalar.activation(rsv[:], ssq_ps[:], AF.Ln,
                                     scale=1.0 / HD, bias=eps_sb[:, 0:1])
                rsv16 = tmp.tile([P, 512], F16, tag="rsv16", bufs=2)
                nc.scalar.activation(rsv16[:], rsv[:], AF.Exp, scale=-0.5)
                if not unit_w:
                    nc.vector.tensor_scalar_mul(
                        rsv16[:], rsv16[:], wqk_sb[:, which:which + 1])
                qn = tmp.tile([P, 512], F16, tag="qn", bufs=2, name="qn")
                nc.vector.tensor_tensor(qn[:], qpre[:], rsv16[:], OP.mult)
                tcos = tmp.tile([P, 512], F16, tag="tcos", bufs=2)
                nc.vector.tensor_tensor(tcos[:], qn[:], tbq[:, 0, tsl],
                                        OP.mult)
                t2 = tmp.tile([P, 512], F16, tag="t2", bufs=2)
                # sin table halves pre-swapped host-side; rotate-half via
                # shifted-partition writes
                nc.vector.tensor_tensor(t2[0:64, :], qn[64:128, :],
                                        tbq[64:128, 1, tsl], OP.mult)
                nc.vector.tensor_tensor(t2[64:128, :], qn[0:64, :],
                                        tbq[0:64, 1, tsl], OP.mult)
                nc.vector.tensor_tensor(dest, tcos[:], t2[:], OP.add)

            def emit_k_proj(j):
                tsl = slice(512 * j, 512 * j + 512)
                mm_ps = psp.tile([P, 512], F32, tag="proj", bufs=2,
                                 name=f"kproj_{j}")
                for dt in range(DT):
                    nc.tensor.matmul(mm_ps[:], wkv_sb[:, dt, 0:128],
                                     hslice(dt, j),
                                     start=(dt == 0), stop=(dt == DT - 1))
                _norm_rope(mm_ps, kTr[:, tsl], tsl, 1)

            def emit_v_proj(j):
                """v projection in [hd, t], then PE-transpose to [t, hd]."""
                v_ps = psp.tile([P, 512], F32, tag="vps", bufs=1,
                                name=f"vproj_{j}")
                for dt in range(DT):
                    nc.tensor.matmul(v_ps[:], wkv_sb[:, dt, 128:256],
                                     hslice(dt, j),
                                     start=(dt == 0), stop=(dt == DT - 1))
                vtmp = tmp.tile([P, 512], F16, tag="vtmp", bufs=2)
                nc.vector.tensor_copy(vtmp[:], v_ps[:])
                for kk in range(4):
                    tt = 4 * j + kk
                    trp = psp.tile([P, P], F16, tag="aux", bufs=1)
                    nc.tensor.transpose(
                        trp[:], vtmp[:, 128 * kk:128 * kk + 128], ident16[:])
                    nc.vector.tensor_copy(v_sb[:, tt, :], trp[:])

            def emit_q_proj(h, j):
                tsl = slice(512 * j, 512 * j + 512)
                mm_ps = psp.tile([P, 512], F32, tag="proj", bufs=2,
                                 name=f"qproj_{h}_{j}")
                for dt in range(DT):
                    nc.tensor.matmul(mm_ps[:], wqg_sb[:, dt, 128 * h:128 * h + 128],
                                     hslice(dt, j),
                                     start=(dt == 0), stop=(dt == DT - 1))
                _norm_rope(mm_ps, qTr[h][:, tsl], tsl, 0)

            def emit_gate_proj(h, j):
                """gate projection -> gT[h] holds ln(1+e^-g)."""
                tsl = slice(512 * j, 512 * j + 512)
                mm_ps = psp.tile([P, 512], F32, tag="proj", bufs=2,
                                 name=f"gproj_{h}_{j}")
                for dt in range(DT):
                    nc.tensor.matmul(
                        mm_ps[:], wqg_sb[:, dt, 256 + 128 * h:384 + 128 * h],
                        hslice(dt, j),
                        start=(dt == 0), stop=(dt == DT - 1))
                eg = tmp.tile([P, 512], F32, tag="eg", bufs=2)
                nc.scalar.activation(eg[:], mm_ps[:], AF.Exp, scale=-1.0)
                nc.scalar.activation(gT[h][:, tsl], eg[:], AF.Ln, bias=1.0)

            def emit_attention(h, j):
                tsl = slice(512 * j, 512 * j + 512)
                valid = [i for i in range(NT) if not flags[i][j][0]]
                last = len(valid) - 1
                ot_ps = psp.tile([P, 512], F32, tag="acc", bufs=1,
                                 name=f"ot_{h}_{j}")
                acc = tmp.tile([P, 512], F16, tag="pacc", bufs=2,
                               name=f"pacc_{h}_{j}")
                for idx, i in enumerate(valid):
                    _, needs_c, needs_s = flags[i][j]
                    st_ps = psp.tile([P, 512], F32, tag="st", bufs=3,
                                     name=f"st_{h}_{j}_{i}")
                    nc.tensor.matmul(st_ps[:], kTr[:, P * i:P * i + P],
                                     qTr[h][:, tsl], start=True, stop=True)
                    pt = ptp.tile([P, 512], F16, tag="pt", name=f"pt_{h}_{j}_{i}")
                    nc.scalar.activation(pt[:], st_ps[:], AF.Exp,
                                         bias=expb_sb[:, 0:1])
                    if needs_c:
                        # causal: keep t >= 128*i - 512*j + s  (per-partition s)
                        nc.vector.scalar_tensor_tensor(
                            out=pt[:], in0=iota_sb[:],
                            scalar=rel_sb[:, 0, i, j:j + 1], in1=pt[:],
                            op0=OP.is_ge, op1=OP.mult)
                    if needs_s:
                        # segment: keep t < seg_end[s] - 512*j
                        nc.vector.scalar_tensor_tensor(
                            out=pt[:], in0=iota_sb[:],
                            scalar=rel_sb[:, 1, i, j:j + 1], in1=pt[:],
                            op0=OP.is_lt, op1=OP.mult)
                    nc.tensor.matmul(ot_ps[:], v_sb[:, i, :], pt[:],
                                     start=(idx == 0), stop=(idx == last))
                    if idx == 0:
                        nc.vector.tensor_copy(acc[:], pt[:])
                    else:
                        nc.vector.tensor_tensor(acc[:], acc[:], pt[:], OP.add)
                rs_ps = psp.tile([P, 512], F32, tag="st", bufs=3,
                                 name=f"rs_{h}_{j}")
                nc.tensor.matmul(rs_ps[:], ones16[:], acc[:],
                                 start=True, stop=True)

                # sig(g)/rowsum = exp(-(ln(1+e^-g) + ln(rowsum)));
                # the exp bias cancels between ot and rs
                sg = tmp.tile([P, 512], F16, tag="sg", bufs=2, name=f"sg_{h}_{j}")
                nc.scalar.activation(sg[:], rs_ps[:], AF.Ln)
                nc.vector.tensor_tensor(sg[:], sg[:], gT[h][:, tsl], OP.add)
                nc.scalar.activation(sg[:], sg[:], AF.Exp, scale=-1.0)
                atg = tmp.tile([P, 512], F16, tag="atg", bufs=2,
                               name=f"atg_{h}_{j}")
                nc.vector.tensor_tensor(atg[:], ot_ps[:], sg[:], OP.mult)
                # one staging DMA: [128, 2, 256] -> dram shards 2j, 2j+1
                nc.sync.dma_start(
                    a2a_in_r[h][:, 2 * j:2 * j + 2, :],
                    atg[:].rearrange("p (s t) -> p s t", t=256))

            # ================= phase A: h0 path =================
            # kv pass pipelines with the hT chunk DMAs; q0/g0/attention pass
            # follows once wqg and the rope tables have landed
            nc.sync.dma_start(wkv_sb[:], wkv_d[:])
            for j in range(NJ):
                t_ = hw.tile([P, DT, 512], F16, tag="hT", bufs=4,
                             name=f"hTc{j}")
                for s in range(2):
                    nc.sync.dma_start(t_[:, 8 * s:8 * s + 8, :],
                                      hT_d[:, 8 * s:8 * s + 8,
                                           512 * j:512 * j + 512])
                hTc.append(t_)
                if j == 1:  # wqg lands just before the q0 pass needs it
                    nc.sync.dma_start(wqg_sb[:], wqg_d[:])
                    nc.sync.dma_start(tbq[:], tblq_d[:])
                    nc.sync.dma_start(iota_sb[:], iota_d[:])
                    nc.sync.dma_start(rel_sb[:], rel_d[:])

            for j in range(NJ):
                emit_k_proj(j)
                emit_v_proj(j)
            for j in range(NJ):
                emit_q_proj(0, j)
                emit_gate_proj(0, j)
                emit_attention(0, j)
            if use_collective:
                nc.gpsimd.collective_compute(
                    "AllToAll", OP.bypass,
                    replica_groups=[list(range(NCORES))],
                    ins=[a2a_in[0][:].opt()], outs=[a2a_out[0][:].opt()])
            else:
                nc.sync.dma_start(a2a_out[0][:], a2a_in[0][:])

            # ================= phase C: h1 path =================
            # wo blocks 0-1 prefetch; rest stream in phase D
            wo_sb = []
            w_ = hw.tile([P, 2, 2048], F16, tag="wo", bufs=4, name="wo_0")
            nc.sync.dma_start(w_[:], wo_d[:, 0:2, :])
            wo_sb.append(w_)
            # gather h0 NOW, before the A2A2 emission: both collectives bump
            # one Collectives sem, so a DMA emitted after A2A2 inherits a
            # cumulative >=2 wait and sits out the whole second collective
            nc.sync.dma_start(ATall[0][:], a2a_out_r[0][:])
            for j in range(NJ):
                emit_q_proj(1, j)
                emit_gate_proj(1, j)
                emit_attention(1, j)
            if use_collective:
                nc.gpsimd.collective_compute(
                    "AllToAll", OP.bypass,
                    replica_groups=[list(range(NCORES))],
                    ins=[a2a_in[1][:].opt()], outs=[a2a_out[1][:].opt()])
            else:
                nc.sync.dma_start(a2a_out[1][:], a2a_in[1][:])

            # ================= phase D: o-proj =================
            # 8 PSUM banks [m 0/1] x [Dc 0..3] accumulate over 16 ht blocks;
            # h0 blocks (ht 0..7) run inside the A2A2 window; within each
            # half m0 completes before m1 so its write overlaps m1 compute
            for w in range(1, 4):
                w_ = hw.tile([P, 2, 2048], F16, tag="wo", bufs=4,
                             name=f"wo_{w}")
                nc.sync.dma_start(w_[:], wo_d[:, 2 * w:2 * w + 2, :])
                wo_sb.append(w_)
            ops_tags = ["proj", "proj", "vps", "st", "st", "st", "acc", "aux"]
            ops_bufs = {"proj": 2, "vps": 1, "st": 3, "acc": 1, "aux": 1}
            ops = []
            for m in range(2):
                for Dc in range(NJ):
                    tg = ops_tags[m * NJ + Dc]
                    ops.append(psp.tile([P, 512], F32, tag=tg,
                                        bufs=ops_bufs[tg],
                                        name=f"ops{m}_{Dc}"))
            for half in range(2):
                for m in range(2):
                    for ht in range(8 * half, 8 * half + 8):
                        at_ap = ATall[half][:, ht % 8,
                                            128 * m:128 * m + 128]
                        for Dc in range(NJ):
                            nc.tensor.matmul(
                                ops[m * NJ + Dc][:], at_ap,
                                wo_sb[ht // 2][:, ht % 2,
                                               512 * Dc:512 * Dc + 512],
                                start=(ht == 0), stop=(ht == NT - 1))
                if half == 0:
                    # emitted only now: an earlier emission shares a DMA
                    # completion lane with the h0 loads and its (A2A2-gated)
                    # count would hold back the half-0 matmul waits; same
                    # for the half-1 wo tiles, which also recycle wo slots
                    nc.sync.dma_start(ATall[1][:], a2a_out_r[1][:])
                    for w in range(4, 8):
                        w_ = hw.tile([P, 2, 2048], F16, tag="wo", bufs=4,
                                     name=f"wo_{w}")
                        nc.sync.dma_start(w_[:], wo_d[:, 2 * w:2 * w + 2, :])
                        wo_sb.append(w_)
            # write m0 while m1's second half still accumulates
            for m in range(2):
                for Dh in range(2):
                    o_sb = tmp.tile([P, 1024], F32, tag="osb", bufs=2,
                                    name=f"o_{m}_{Dh}")
                    for q in range(2):
                        nc.vector.tensor_copy(o_sb[:, 512 * q:512 * q + 512],
                                              ops[m * NJ + 2 * Dh + q][:])
                    nc.sync.dma_start(
                        out_d[128 * m:128 * m + 128,
                              1024 * Dh:1024 * Dh + 1024], o_sb[:])

    nc.compile()
    _dedupe_act_table_loads(nc)
    return nc


def _dedupe_act_table_loads(nc):
    """Bacc assigns Exp->exp_and_others and Ln->natural_log, inserting a
    ~2.7us table load at every Exp<->Ln alternation. All activation funcs
    this kernel uses (Exp, Ln, Square) live in the natural_log_exp_and_others
    set, so keep one load of that set and drop the rest."""
    from concourse.hw_specs import get_activation_tables
    tabs = list(get_activation_tables(nc.m.arch).items())
    nl_exp = next(i for i, (nm, funcs) in enumerate(tabs)
                  if nm == "natural_log_exp_and_others")
    used = {ins.func for bb in nc.main_func.blocks for ins in bb.instructions
            if isinstance(ins, mybir.InstActivation)}
    assert used <= tabs[nl_exp][1], f"funcs {used} not all in natural_log_exp"
    first = True
    for bb in nc.main_func.blocks:
        keep = []
        for ins in bb.instructions:
            if isinstance(ins, mybir.InstLoadActFuncSet):
                assert ins.sync_info is None or (
                    not ins.sync_info.on_wait and not ins.sync_info.on_update)
                if first:
                    ins.act_func_set_id = nl_exp
                    keep.append(ins)
                    first = False
                continue
            keep.append(ins)
        bb.instructions[:] = keep


def _host_prep(hidden_BTD, cos_BTK, sin_BTK, segment_ids_BT, position_ids_BT,
               wq, wk, wv, wo, q_norm_w, k_norm_w):
    hidden = np.ascontiguousarray(np.asarray(hidden_BTD, dtype=np.float32)[0])
    cos = np.asarray(cos_BTK, dtype=np.float32)[0]
    sin = np.asarray(sin_BTK, dtype=np.float32)[0]
    seg = np.asarray(segment_ids_BT)[0]
    pos = np.asarray(position_ids_BT)[0]
    wq = np.asarray(wq, dtype=np.float32)
    wk = np.asarray(wk, dtype=np.float32)
    wv = np.asarray(wv, dtype=np.float32)
    wo = np.asarray(wo, dtype=np.float32)
    q_norm_w = np.asarray(q_norm_w, dtype=np.float32)
    k_norm_w = np.asarray(k_norm_w, dtype=np.float32)

    assert np.array_equal(pos, np.arange(T, dtype=pos.dtype)), \
        "kernel assumes position_ids == arange"
    assert np.all(np.diff(seg) >= 0), "kernel assumes sorted segment ids"

    # [P, DT, T] fp16 partition-major hidden^T
    hT = np.ascontiguousarray(
        hidden.T.reshape(DT, P, T).transpose(1, 0, 2).astype(np.float16))
    sqrtS = np.float32(np.sqrt(SCALE))
    signv = np.where(np.arange(HD) < HD // 2, -1.0, 1.0).astype(np.float32)
    shuf = (np.arange(HD) + HD // 2) % HD

    cosw = (cos.T * sqrtS).astype(np.float32)
    sinw = (sin.T * signv[:, None] * sqrtS).astype(np.float32)
    sinswap = sinw[shuf]  # halves swapped: see rotate-half ops
    tblq = np.ascontiguousarray(
        np.stack([cosw, sinswap], axis=1).astype(np.float16))  # [P, 2, T]
    unit_w = bool(np.all(q_norm_w == 1.0) and np.all(k_norm_w == 1.0))
    wqk = np.ascontiguousarray(np.stack([q_norm_w, k_norm_w], axis=1))

    # prepack wo; ht-block order: all h0 head-blocks, then all h1
    perm = [2 * i + h for h in range(2) for i in range(NCORES)]
    wo_p = wo.reshape(NT, P, 2048)[perm].transpose(1, 0, 2)
    wo_p = np.ascontiguousarray(wo_p.astype(np.float16))

    seg_end = np.searchsorted(seg, seg, side="right").astype(np.int64)
    iota = np.broadcast_to(np.arange(512, dtype=np.float16), (P, 512)).copy()
    # rel[:, 0] causal: t-iota >= 128i - 512j + s; rel[:, 1] segment:
    # t-iota < seg_end[s] - 512j
    rel = np.zeros((P, 2, NT, NJ), dtype=np.float32)
    s_local = np.arange(P, dtype=np.float32)
    for i in range(NT):
        for j in range(NJ):
            rel[:, 0, i, j] = 128.0 * i - 512.0 * j + s_local
            rel[:, 1, i, j] = seg_end[P * i:P * i + P] - 512.0 * j

    in_maps = []
    for c in range(NCORES):
        h0, h1 = 2 * c, 2 * c + 1
        g = c // 2
        # per-core wq/gate pack: [q_h0 | q_h1 | g_h0 | g_h1] columns
        wqg = np.concatenate([
            wq[:, h0 * 256: h0 * 256 + 128],
            wq[:, h1 * 256: h1 * 256 + 128],
            wq[:, h0 * 256 + 128: h0 * 256 + 256],
            wq[:, h1 * 256 + 128: h1 * 256 + 256],
        ], axis=1)
        wqg_p = np.ascontiguousarray(
            wqg.reshape(DT, P, 512).transpose(1, 0, 2).astype(np.float16))
        wkv = np.concatenate([
            wk[:, g * 128:(g + 1) * 128], wv[:, g * 128:(g + 1) * 128]], axis=1)
        wkv_p = np.ascontiguousarray(
            wkv.reshape(DT, P, 256).transpose(1, 0, 2).astype(np.float16))
        m = {
            "hT": hT, "wqg": wqg_p, "wkv": wkv_p, "wo": wo_p,
            "tblq": tblq, "iota": iota, "rel": rel,
        }
        if not unit_w:
            m["wqk"] = wqk
        in_maps.append(m)
    return in_maps, seg_end, unit_w


def kernel(**inputs) -> np.ndarray:
    in_maps, seg_end, unit_w = _host_prep(**inputs)
    key = (_tile_flags(seg_end), unit_w)
    if key not in _program_cache:
        _program_cache[key] = _build_program(key)
    nc = _program_cache[key]
    res = run_bass_kernel_spmd(nc, in_maps, list(range(NCORES)))
    out = np.concatenate([res.results[c]["out"] for c in range(NCORES)], axis=0)
    return out[None].astype(np.float32)
